# revision 6
# baseline (speedup 1.0000x reference)
"""ChebConv GNN (K=3, 3 layers) distributed Bass kernel for 8 NeuronCores.

kernel(**inputs) takes FULL numpy inputs (as in setup_inputs) and returns
the FULL [N, 40] float32 log_softmax output.

Design:
- Nodes sharded contiguously across 8 cores (12544 padded rows/core).
- SpMM via dma_gather from a replicated HBM feature table: per core,
  edges bucketed into 4 col-banks (25088-row int16 windows); within a
  bank, local rows are sorted by bank-degree and edges arranged in
  slabs (k-th bank-edge per row) so gather position == accumulator
  slot; per-edge norm applied via DVE broadcast-mul; slab adds on DVE;
  bank partials recombined with static-index gathers + adds.
- Slab schedule padded to a core-uniform profile (SPMD: one program).
- AllGather refreshes the replicated table after each SpMM producer.
- Dense 64x64 matmuls run feature-major on TensorE via PE transposes;
  bias+ReLU on ScalarE; log_softmax fused wide on DVE/ACT.
"""

import numpy as np

import concourse.bacc as bacc
import concourse.mybir as mybir
import concourse.tile as tile
from concourse.bass_utils import run_bass_kernel_spmd

C = 8            # cores
P = 128
SROWS = 12544    # rows per core (98 * 128)
NBANK = 4
BROWS = 2 * SROWS          # 25088-row bank window (< 32768 for int16 idx)
TROWS = C * SROWS          # padded table rows = 100352
N_REAL = 100000
F_IN = 64
HID = 64
F_OUT_REAL = 40
NT = SROWS // P  # 98 node tiles per core
MAXCALL = 4096   # idxs per dma_gather call

TRACE = [False]
LAST_EXEC_NS = [None]
_CACHE = {}


def _wrap_idx(idx):
    """dma_gather idx layout [128, len/16] int16: position j ->
    (partition j%16, slot j//16), replicated across 8 Q7 core groups."""
    n = len(idx)
    a = idx.astype(np.int16).reshape(n // 16, 16).T
    return np.broadcast_to(a[None], (8, 16, n // 16)).reshape(P, n // 16)


def _host_prep(edge_index, edge_attr):
    row = edge_index[0].astype(np.int64)
    col = edge_index[1].astype(np.int64)
    w = edge_attr.astype(np.float64)
    deg = np.zeros(N_REAL)
    np.add.at(deg, row, w)
    dinv = np.where(deg > 0, deg ** -0.5, 0.0)
    norm = (-(dinv[row] * w * dinv[col])).astype(np.float32)

    per = [[None] * NBANK for _ in range(C)]
    shard = row // SROWS
    bank = col // BROWS
    for c in range(C):
        mc = shard == c
        for b in range(NBANK):
            m = mc & (bank == b)
            er = row[m] - c * SROWS
            ec = col[m] - b * BROWS
            en = norm[m]
            bdeg = np.bincount(er, minlength=SROWS)
            order = np.argsort(-bdeg, kind="stable")   # slot -> row
            rank = np.empty(SROWS, dtype=np.int64)     # row -> slot
            rank[order] = np.arange(SROWS)
            sdeg = bdeg[order]
            maxd = int(sdeg[0]) if len(er) else 0
            lens = [int((sdeg > k).sum()) for k in range(maxd)]
            eslot = rank[er]
            o1 = np.argsort(eslot, kind="stable")
            es = eslot[o1]
            kidx = np.arange(len(es)) - np.searchsorted(es, es)
            o2 = np.lexsort((es, kidx))
            eorder = o1[o2]
            # edges now ordered (k, slot); within slab k, position = slot
            per[c][b] = dict(lens=lens, eslot=eslot[eorder], ecol=ec[eorder],
                             enorm=en[eorder], rank=rank)

    profile = []
    for b in range(NBANK):
        nk = max(len(per[c][b]["lens"]) for c in range(C))
        plens = []
        for k in range(nk):
            L = max((per[c][b]["lens"][k] if k < len(per[c][b]["lens"]) else 0)
                    for c in range(C))
            plens.append(max(P, -(-L // P) * P))
        profile.append(plens)
    totpos = sum(sum(pl) for pl in profile)

    gidx = np.zeros((C, P, totpos // 16), dtype=np.int16)
    gnorm = np.zeros((C, P, totpos // P), dtype=np.float32)
    # call = (bank, idx16_off, gnorm_slot_off, acc_slot_off, num_idx)
    calls = []
    off = 0
    for b in range(NBANK):
        for k, L in enumerate(profile[b]):
            pos0 = off
            for c in range(C):
                d = per[c][b]
                idx = np.zeros(L, dtype=np.int64)
                nrm = np.zeros(L, dtype=np.float32)
                if k < len(d["lens"]):
                    lk = d["lens"][k]
                    s0 = sum(d["lens"][:k])
                    sl = d["eslot"][s0:s0 + lk]
                    idx[sl] = d["ecol"][s0:s0 + lk]
                    nrm[sl] = d["enorm"][s0:s0 + lk]
                gnorm[c][:, pos0 // P:(pos0 + L) // P] = nrm.reshape(L // P, P).T
                o = pos0
                for cs in range(0, L, MAXCALL):
                    ni = min(MAXCALL, L - cs)
                    gidx[c][:, o // 16:(o + ni) // 16] = _wrap_idx(idx[cs:cs + ni])
                    o += ni
            for cs in range(0, L, MAXCALL):
                ni = min(MAXCALL, L - cs)
                calls.append((b, (pos0 + cs) // 16, (pos0 + cs) // P, cs // P, ni))
            off += L

    ridx = np.zeros((C, NBANK, P, SROWS // 16), dtype=np.int16)
    rcalls = []
    for b in range(NBANK):
        for c in range(C):
            rk = per[c][b]["rank"]
            for cs in range(0, SROWS, MAXCALL):
                ni = min(MAXCALL, SROWS - cs)
                ridx[c][b][:, cs // 16:(cs + ni) // 16] = _wrap_idx(rk[cs:cs + ni])
    for cs in range(0, SROWS, MAXCALL):
        rcalls.append((cs // 16, cs // P, min(MAXCALL, SROWS - cs)))
    return dict(gidx=gidx, gnorm=gnorm, ridx=ridx, calls=calls, rcalls=rcalls,
                totpos=totpos)


def _build(prep):
    totpos = prep["totpos"]
    calls = prep["calls"]
    rcalls = prep["rcalls"]
    f32 = mybir.dt.float32
    i16 = mybir.dt.int16
    AO = mybir.AluOpType

    nc = bacc.Bacc("TRN2", target_bir_lowering=False, debug=False, num_devices=C)
    x_own = nc.declare_dram_parameter("x_own", [SROWS, F_IN], f32, isOutput=False)
    x_table = nc.declare_dram_parameter("x_table", [TROWS, F_IN], f32, isOutput=False)
    gidx_d = nc.declare_dram_parameter("gidx", [P, totpos // 16], i16, isOutput=False)
    gnorm_d = nc.declare_dram_parameter("gnorm", [P, totpos // P], f32, isOutput=False)
    ridx_d = nc.declare_dram_parameter("ridx", [NBANK, P, SROWS // 16], i16, isOutput=False)
    Wd, bd = [], []
    for i in range(3):
        Wd.append(nc.declare_dram_parameter(f"W{i}", [3, HID, HID], f32, isOutput=False))
        bd.append(nc.declare_dram_parameter(f"b{i}", [HID], f32, isOutput=False))
    yout = nc.declare_dram_parameter("yout", [SROWS, HID], f32, isOutput=True)

    accb = nc.dram_tensor("accb", [NBANK, SROWS, HID], f32)
    agin = [nc.dram_tensor(f"agin{i}", [SROWS, HID], f32) for i in range(5)]
    agout = [nc.dram_tensor(f"agout{i}", [TROWS, HID], f32, addr_space="Shared")
             for i in range(5)]

    with tile.TileContext(nc) as tc:
        with (
            tc.tile_pool(name="res", bufs=1) as res,
            tc.tile_pool(name="stage", bufs=2) as stagep,
            tc.tile_pool(name="accp", bufs=1) as accp,
            tc.tile_pool(name="idxp", bufs=3) as idxp,
            tc.tile_pool(name="small", bufs=4) as smallp,
            tc.tile_pool(name="psum", bufs=2, space="PSUM") as psump,
            tc.tile_pool(name="txp", bufs=1) as txp,
        ):
            from concourse.masks import make_identity
            ident = res.tile([P, P], f32)
            make_identity(nc, ident[:])
            gnorm_t = res.tile([P, totpos // P], f32)
            nc.sync.dma_start(out=gnorm_t[:], in_=gnorm_d[:, :])
            Wt, bt = [], []
            for i in range(3):
                ws = []
                for k in range(3):
                    t = res.tile([HID, HID], f32, tag=f"w{i}{k}")
                    nc.sync.dma_start(out=t[:], in_=Wd[i][k])
                    ws.append(t)
                Wt.append(ws)
                t = res.tile([HID, 1], f32, tag=f"bb{i}")
                nc.sync.dma_start(out=t[:], in_=bd[i][:, None])
                bt.append(t)

            tx0 = txp.tile([P, NT, HID], f32, tag="tx0")
            tx1 = txp.tile([P, NT, HID], f32, tag="tx1")
            tx2 = txp.tile([P, NT, HID], f32, tag="tx2")
            nc.sync.dma_start(out=tx0[:], in_=x_own.ap().rearrange("(a p) d -> p a d", p=P))

            def spmm(table_d, out_tile):
                """out_tile[128, NT, HID] node-major canonical = L @ table."""
                for b in range(NBANK):
                    acc = accp.tile([P, NT, HID], f32, tag="acc")
                    nc.vector.memset(acc[:], 0.0)
                    for (bb, i16o, so, ao, ni) in calls:
                        if bb != b:
                            continue
                        st = stagep.tile([P, MAXCALL // P, HID], f32, tag="st")
                        it = idxp.tile([P, MAXCALL // 16], i16, tag="it")
                        nc.sync.dma_start(out=it[:, :ni // 16],
                                          in_=gidx_d[:, i16o:i16o + ni // 16])
                        nc.gpsimd.dma_gather(
                            st[:, :ni // P, :],
                            table_d[b * BROWS:(b + 1) * BROWS, :],
                            it[:, :ni // 16], ni, ni, HID,
                            single_packet=False,
                        )
                        nrm_b = gnorm_t[:, so:so + ni // P, None].to_broadcast(
                            [P, ni // P, HID])
                        nc.vector.tensor_tensor(out=st[:, :ni // P, :],
                                                in0=st[:, :ni // P, :],
                                                in1=nrm_b, op=AO.mult)
                        nc.vector.tensor_tensor(
                            out=acc[:, ao:ao + ni // P, :],
                            in0=acc[:, ao:ao + ni // P, :],
                            in1=st[:, :ni // P, :], op=AO.add)
                    nc.sync.dma_start(
                        out=accb.ap()[b].rearrange("(a p) d -> p a d", p=P),
                        in_=acc[:])
                for b in range(NBANK):
                    for (i16o, so, ni) in rcalls:
                        it = idxp.tile([P, MAXCALL // 16], i16, tag="it")
                        nc.sync.dma_start(out=it[:, :ni // 16],
                                          in_=ridx_d[b][:, i16o:i16o + ni // 16])
                        rst = stagep.tile([P, MAXCALL // P, HID], f32, tag="st")
                        nc.gpsimd.dma_gather(
                            rst[:, :ni // P, :], accb.ap()[b],
                            it[:, :ni // 16], ni, ni, HID,
                            single_packet=False,
                        )
                        if b == 0:
                            nc.vector.tensor_copy(out=out_tile[:, so:so + ni // P, :],
                                                  in_=rst[:, :ni // P, :])
                        else:
                            nc.vector.tensor_tensor(
                                out=out_tile[:, so:so + ni // P, :],
                                in0=out_tile[:, so:so + ni // P, :],
                                in1=rst[:, :ni // P, :], op=AO.add)

            def allgather(src_tile, i):
                nc.sync.dma_start(
                    out=agin[i].ap().rearrange("(a p) d -> p a d", p=P),
                    in_=src_tile[:])
                nc.gpsimd.collective_compute(
                    "AllGather", AO.bypass,
                    replica_groups=[list(range(C))],
                    ins=[agin[i].ap().opt()], outs=[agout[i].ap().opt()],
                )
                return agout[i].ap()

            def transpose_tile(src):  # [128, 64] sbuf -> [64, 128] sbuf
                pt = psump.tile([HID, P], f32, tag="tp")
                nc.tensor.transpose(out=pt[:], in_=src, identity=ident[:])
                st = smallp.tile([HID, P], f32, tag="tps")
                nc.scalar.copy(out=st[:], in_=pt[:])
                return st

            def dense(txs, li, out_tile, is_last):
                """out_tile[128, NT, HID] = act(sum_k txs[k] @ W[li][k] + b)."""
                for t in range(NT):
                    tts = [transpose_tile(tx[:, t, :]) for tx in txs]
                    pm = psump.tile([HID, P], f32, tag="mm")
                    for k in range(3):
                        nc.tensor.matmul(pm[:], Wt[li][k][:], tts[k][:],
                                         start=(k == 0), stop=(k == 2))
                    ot = smallp.tile([HID, P], f32, tag="ot")
                    nc.scalar.activation(ot[:], pm[:],
                                         mybir.ActivationFunctionType.Relu,
                                         bias=bt[li][:])
                    # transpose back to node-major
                    pt2 = psump.tile([P, HID], f32, tag="tb")
                    nc.tensor.transpose(out=pt2[:], in_=ot[:],
                                        identity=ident[:HID, :HID])
                    nc.scalar.copy(out=out_tile[:, t, :], in_=pt2[:])

            table = x_table.ap()
            agi = 0
            for li in range(3):
                is_last = li == 2
                spmm(table, tx1)
                t1tab = allgather(tx1, agi); agi += 1
                spmm(t1tab, tx2)
                # tx2 = 2*L(tx1) - tx0
                nc.vector.tensor_scalar_mul(tx2[:], tx2[:], 2.0)
                nc.vector.tensor_tensor(out=tx2[:], in0=tx2[:], in1=tx0[:],
                                        op=AO.subtract)
                dense([tx0, tx1, tx2], li, tx0, is_last)
                if not is_last:
                    table = allgather(tx0, agi); agi += 1

            # log_softmax over first F_OUT_REAL features (rest of HID cols
            # are padding: W2/b2 padded with -inf? -> handled by masking)
            lg = tx0
            mx = smallp.tile([P, NT, 1], f32, tag="mx")
            nc.vector.tensor_reduce(out=mx[:], in_=lg[:, :, :F_OUT_REAL],
                                    axis=mybir.AxisListType.X, op=AO.max)
            sh = txp.tile([P, NT, F_OUT_REAL], f32, tag="sh")
            nc.vector.tensor_tensor(
                out=sh[:], in0=lg[:, :, :F_OUT_REAL],
                in1=mx[:].to_broadcast([P, NT, F_OUT_REAL]), op=AO.subtract)
            ex = txp.tile([P, NT, F_OUT_REAL], f32, tag="ex")
            nc.scalar.activation(ex[:], sh[:],
                                 mybir.ActivationFunctionType.Exp)
            sm = smallp.tile([P, NT, 1], f32, tag="sm")
            nc.vector.tensor_reduce(out=sm[:], in_=ex[:],
                                    axis=mybir.AxisListType.X, op=AO.add)
            lz = smallp.tile([P, NT, 1], f32, tag="lz")
            nc.scalar.activation(lz[:], sm[:], mybir.ActivationFunctionType.Ln)
            nc.vector.tensor_tensor(
                out=sh[:], in0=sh[:],
                in1=lz[:].to_broadcast([P, NT, F_OUT_REAL]), op=AO.subtract)
            out_t = tx1
            nc.vector.memset(out_t[:], 0.0)
            nc.vector.tensor_copy(out=out_t[:, :, :F_OUT_REAL], in_=sh[:])
            nc.sync.dma_start(
                out=yout.ap().rearrange("(a p) d -> p a d", p=P), in_=out_t[:])
    nc.compile()
    return nc


def kernel(x, edge_index, edge_attr, W0, b0, W1, b1, W2, b2):
    x = np.asarray(x)
    edge_index = np.asarray(edge_index)
    edge_attr = np.asarray(edge_attr)
    key = hash((edge_index.tobytes(), edge_attr.tobytes()))
    if key in _CACHE:
        nc, prep = _CACHE[key]
    else:
        prep = _host_prep(edge_index, edge_attr)
        nc = _build(prep)
        _CACHE[key] = (nc, prep)

    # pad weights/bias to HID=64 wide
    W2p = np.zeros((3, HID, HID), dtype=np.float32)
    W2p[:, :, :F_OUT_REAL] = np.asarray(W2, dtype=np.float32)
    b2p = np.zeros((HID,), dtype=np.float32)
    b2p[:F_OUT_REAL] = np.asarray(b2, dtype=np.float32)

    xpad = np.zeros((TROWS, F_IN), dtype=np.float32)
    xpad[:N_REAL] = np.asarray(x, dtype=np.float32)

    in_maps = []
    for c in range(C):
        in_maps.append({
            "x_own": xpad[c * SROWS:(c + 1) * SROWS],
            "x_table": xpad,
            "gidx": prep["gidx"][c],
            "gnorm": prep["gnorm"][c],
            "ridx": prep["ridx"][c],
            "W0": np.asarray(W0, dtype=np.float32),
            "b0": np.asarray(b0, dtype=np.float32),
            "W1": np.asarray(W1, dtype=np.float32),
            "b1": np.asarray(b1, dtype=np.float32),
            "W2": W2p, "b2": b2p,
        })
    res = run_bass_kernel_spmd(nc, in_maps, core_ids=list(range(C)),
                               trace=TRACE[0])
    LAST_EXEC_NS[0] = res.exec_time_ns
    out = np.concatenate([res.results[c]["yout"] for c in range(C)], axis=0)
    return out[:N_REAL, :F_OUT_REAL].astype(np.float32)


# revision 7
# speedup vs baseline: 1.9562x; 1.9562x over previous
"""ChebConv GNN (K=3, 3 layers) distributed Bass kernel for 8 NeuronCores.

kernel(**inputs) takes FULL numpy inputs (as in setup_inputs) and returns
the FULL [N, 40] float32 log_softmax output.

Design:
- Nodes sharded contiguously across 8 cores (12544 padded rows/core).
- SpMM via dma_gather from a replicated HBM feature table: per core,
  edges bucketed into 4 col-banks (25088-row int16 windows); within a
  bank, local rows are sorted by bank-degree and edges arranged in
  slabs (k-th bank-edge per row) so gather position == accumulator
  slot; per-edge norm applied via DVE broadcast-mul; slab adds on DVE;
  bank partials recombined with static-index gathers + adds.
- Slab schedule padded to a core-uniform profile (SPMD: one program).
- AllGather refreshes the replicated table after each SpMM producer.
- Dense 64x64 matmuls run feature-major on TensorE via PE transposes;
  bias+ReLU on ScalarE; log_softmax fused wide on DVE/ACT.
"""

import numpy as np

import concourse.bacc as bacc
import concourse.mybir as mybir
import concourse.tile as tile
from concourse.bass_utils import run_bass_kernel_spmd

C = 8            # cores
P = 128
SROWS = 12544    # rows per core (98 * 128)
NBANK = 4
BROWS = 2 * SROWS          # 25088-row bank window (< 32768 for int16 idx)
TROWS = C * SROWS          # padded table rows = 100352
N_REAL = 100000
F_IN = 64
HID = 64
F_OUT_REAL = 40
NT = SROWS // P  # 98 node tiles per core
MAXCALL = 4096   # idxs per dma_gather call

TRACE = [False]
LAST_EXEC_NS = [None]
_CACHE = {}


def _wrap_idx(idx):
    """dma_gather idx layout [128, len/16] int16: position j ->
    (partition j%16, slot j//16), replicated across 8 Q7 core groups."""
    n = len(idx)
    a = idx.astype(np.int16).reshape(n // 16, 16).T
    return np.broadcast_to(a[None], (8, 16, n // 16)).reshape(P, n // 16)


def _host_prep(edge_index, edge_attr):
    row = edge_index[0].astype(np.int64)
    col = edge_index[1].astype(np.int64)
    w = edge_attr.astype(np.float64)
    deg = np.zeros(N_REAL)
    np.add.at(deg, row, w)
    dinv = np.where(deg > 0, deg ** -0.5, 0.0)
    norm = (-(dinv[row] * w * dinv[col])).astype(np.float32)

    per = [[None] * NBANK for _ in range(C)]
    shard = row // SROWS
    bank = col // BROWS
    for c in range(C):
        mc = shard == c
        for b in range(NBANK):
            m = mc & (bank == b)
            er = row[m] - c * SROWS
            ec = col[m] - b * BROWS
            en = norm[m]
            bdeg = np.bincount(er, minlength=SROWS)
            order = np.argsort(-bdeg, kind="stable")   # slot -> row
            rank = np.empty(SROWS, dtype=np.int64)     # row -> slot
            rank[order] = np.arange(SROWS)
            sdeg = bdeg[order]
            maxd = int(sdeg[0]) if len(er) else 0
            lens = [int((sdeg > k).sum()) for k in range(maxd)]
            eslot = rank[er]
            o1 = np.argsort(eslot, kind="stable")
            es = eslot[o1]
            kidx = np.arange(len(es)) - np.searchsorted(es, es)
            o2 = np.lexsort((es, kidx))
            eorder = o1[o2]
            # edges now ordered (k, slot); within slab k, position = slot
            per[c][b] = dict(lens=lens, eslot=eslot[eorder], ecol=ec[eorder],
                             enorm=en[eorder], rank=rank)

    profile = []
    for b in range(NBANK):
        nk = max(len(per[c][b]["lens"]) for c in range(C))
        plens = []
        for k in range(nk):
            L = max((per[c][b]["lens"][k] if k < len(per[c][b]["lens"]) else 0)
                    for c in range(C))
            plens.append(max(P, -(-L // P) * P))
        profile.append(plens)
    totpos = sum(sum(pl) for pl in profile)

    gidx = np.zeros((C, P, totpos // 16), dtype=np.int16)
    gnorm = np.zeros((C, P, totpos // P), dtype=np.float32)
    # call = (bank, idx16_off, gnorm_slot_off, acc_slot_off, num_idx)
    calls = []
    off = 0
    for b in range(NBANK):
        for k, L in enumerate(profile[b]):
            pos0 = off
            for c in range(C):
                d = per[c][b]
                idx = np.zeros(L, dtype=np.int64)
                nrm = np.zeros(L, dtype=np.float32)
                if k < len(d["lens"]):
                    lk = d["lens"][k]
                    s0 = sum(d["lens"][:k])
                    sl = d["eslot"][s0:s0 + lk]
                    idx[sl] = d["ecol"][s0:s0 + lk]
                    nrm[sl] = d["enorm"][s0:s0 + lk]
                gnorm[c][:, pos0 // P:(pos0 + L) // P] = nrm.reshape(L // P, P).T
                o = pos0
                for cs in range(0, L, MAXCALL):
                    ni = min(MAXCALL, L - cs)
                    gidx[c][:, o // 16:(o + ni) // 16] = _wrap_idx(idx[cs:cs + ni])
                    o += ni
            for cs in range(0, L, MAXCALL):
                ni = min(MAXCALL, L - cs)
                calls.append((b, (pos0 + cs) // 16, (pos0 + cs) // P, cs // P, ni))
            off += L

    ridx = np.zeros((C, NBANK, P, SROWS // 16), dtype=np.int16)
    rcalls = []
    for b in range(NBANK):
        for c in range(C):
            rk = per[c][b]["rank"]
            for cs in range(0, SROWS, MAXCALL):
                ni = min(MAXCALL, SROWS - cs)
                ridx[c][b][:, cs // 16:(cs + ni) // 16] = _wrap_idx(rk[cs:cs + ni])
    for cs in range(0, SROWS, MAXCALL):
        rcalls.append((cs // 16, cs // P, min(MAXCALL, SROWS - cs)))
    return dict(gidx=gidx, gnorm=gnorm, ridx=ridx, calls=calls, rcalls=rcalls,
                totpos=totpos)


def _build(prep):
    totpos = prep["totpos"]
    calls = prep["calls"]
    rcalls = prep["rcalls"]
    f32 = mybir.dt.float32
    i16 = mybir.dt.int16
    AO = mybir.AluOpType

    nc = bacc.Bacc("TRN2", target_bir_lowering=False, debug=False, num_devices=C,
                   num_swdge_queues=4)
    x_own = nc.declare_dram_parameter("x_own", [SROWS, F_IN], f32, isOutput=False)
    x_table = nc.declare_dram_parameter("x_table", [TROWS, F_IN], f32, isOutput=False)
    gidx_d = nc.declare_dram_parameter("gidx", [P, totpos // 16], i16, isOutput=False)
    gnorm_d = nc.declare_dram_parameter("gnorm", [P, totpos // P], f32, isOutput=False)
    ridx_d = nc.declare_dram_parameter("ridx", [NBANK, P, SROWS // 16], i16, isOutput=False)
    Wd, bd = [], []
    for i in range(3):
        Wd.append(nc.declare_dram_parameter(f"W{i}", [3, HID, HID], f32, isOutput=False))
        bd.append(nc.declare_dram_parameter(f"b{i}", [HID], f32, isOutput=False))
    yout = nc.declare_dram_parameter("yout", [SROWS, HID], f32, isOutput=True)

    accb = nc.dram_tensor("accb", [NBANK, SROWS, HID], f32)
    agin = [nc.dram_tensor(f"agin{i}", [SROWS, HID], f32) for i in range(5)]
    agout = [nc.dram_tensor(f"agout{i}", [TROWS, HID], f32, addr_space="Shared")
             for i in range(5)]

    with tile.TileContext(nc) as tc:
        with (
            tc.tile_pool(name="res", bufs=1) as res,
            tc.tile_pool(name="stage", bufs=4) as stagep,
            tc.tile_pool(name="accp", bufs=1) as accp,
            tc.tile_pool(name="idxp", bufs=6) as idxp,
            tc.tile_pool(name="small", bufs=4) as smallp,
            tc.tile_pool(name="psum", bufs=2, space="PSUM") as psump,
            tc.tile_pool(name="txp", bufs=1) as txp,
        ):
            from concourse.masks import make_identity
            ident = res.tile([P, P], f32)
            make_identity(nc, ident[:])
            gnorm_t = res.tile([P, totpos // P], f32)
            nc.sync.dma_start(out=gnorm_t[:], in_=gnorm_d[:, :])
            Wt, bt = [], []
            for i in range(3):
                ws = []
                for k in range(3):
                    t = res.tile([HID, HID], f32, tag=f"w{i}{k}")
                    nc.sync.dma_start(out=t[:], in_=Wd[i][k])
                    ws.append(t)
                Wt.append(ws)
                t = res.tile([HID, 1], f32, tag=f"bb{i}")
                nc.sync.dma_start(out=t[:], in_=bd[i][:, None])
                bt.append(t)

            tx0 = txp.tile([P, NT, HID], f32, tag="tx0")
            tx1 = txp.tile([P, NT, HID], f32, tag="tx1")
            tx2 = txp.tile([P, NT, HID], f32, tag="tx2")
            nc.sync.dma_start(out=tx0[:], in_=x_own.ap().rearrange("(a p) d -> p a d", p=P))

            qctr = [0]

            def spmm(table_d, out_tile):
                """out_tile[128, NT, HID] node-major canonical = L @ table."""
                for b in range(NBANK):
                    acc = accp.tile([P, NT, HID], f32, tag="acc")
                    nc.vector.memset(acc[:], 0.0)
                    for (bb, i16o, so, ao, ni) in calls:
                        if bb != b:
                            continue
                        st = stagep.tile([P, MAXCALL // P, HID], f32, tag="st")
                        it = idxp.tile([P, MAXCALL // 16], i16, tag="it")
                        nc.sync.dma_start(out=it[:, :ni // 16],
                                          in_=gidx_d[:, i16o:i16o + ni // 16])
                        nc.gpsimd.dma_gather(
                            st[:, :ni // P, :],
                            table_d[b * BROWS:(b + 1) * BROWS, :],
                            it[:, :ni // 16], ni, ni, HID,
                            single_packet=False,
                            queue_num=qctr[0] % 4,
                        )
                        qctr[0] += 1
                        nrm_b = gnorm_t[:, so:so + ni // P, None].to_broadcast(
                            [P, ni // P, HID])
                        nc.vector.tensor_tensor(out=st[:, :ni // P, :],
                                                in0=st[:, :ni // P, :],
                                                in1=nrm_b, op=AO.mult)
                        nc.vector.tensor_tensor(
                            out=acc[:, ao:ao + ni // P, :],
                            in0=acc[:, ao:ao + ni // P, :],
                            in1=st[:, :ni // P, :], op=AO.add)
                    nc.sync.dma_start(
                        out=accb.ap()[b].rearrange("(a p) d -> p a d", p=P),
                        in_=acc[:])
                for b in range(NBANK):
                    for (i16o, so, ni) in rcalls:
                        it = idxp.tile([P, MAXCALL // 16], i16, tag="it")
                        nc.sync.dma_start(out=it[:, :ni // 16],
                                          in_=ridx_d[b][:, i16o:i16o + ni // 16])
                        rst = stagep.tile([P, MAXCALL // P, HID], f32, tag="st")
                        nc.gpsimd.dma_gather(
                            rst[:, :ni // P, :], accb.ap()[b],
                            it[:, :ni // 16], ni, ni, HID,
                            single_packet=False,
                            queue_num=qctr[0] % 4,
                        )
                        qctr[0] += 1
                        if b == 0:
                            nc.vector.tensor_copy(out=out_tile[:, so:so + ni // P, :],
                                                  in_=rst[:, :ni // P, :])
                        else:
                            nc.vector.tensor_tensor(
                                out=out_tile[:, so:so + ni // P, :],
                                in0=out_tile[:, so:so + ni // P, :],
                                in1=rst[:, :ni // P, :], op=AO.add)

            def allgather(src_tile, i):
                nc.sync.dma_start(
                    out=agin[i].ap().rearrange("(a p) d -> p a d", p=P),
                    in_=src_tile[:])
                nc.gpsimd.collective_compute(
                    "AllGather", AO.bypass,
                    replica_groups=[list(range(C))],
                    ins=[agin[i].ap().opt()], outs=[agout[i].ap().opt()],
                )
                return agout[i].ap()

            def transpose_tile(src):  # [128, 64] sbuf -> [64, 128] sbuf
                pt = psump.tile([HID, P], f32, tag="tp")
                nc.tensor.transpose(out=pt[:], in_=src, identity=ident[:])
                st = smallp.tile([HID, P], f32, tag="tps")
                nc.scalar.copy(out=st[:], in_=pt[:])
                return st

            def dense(txs, li, out_tile, is_last):
                """out_tile[128, NT, HID] = act(sum_k txs[k] @ W[li][k] + b)."""
                for t in range(NT):
                    tts = [transpose_tile(tx[:, t, :]) for tx in txs]
                    pm = psump.tile([HID, P], f32, tag="mm")
                    for k in range(3):
                        nc.tensor.matmul(pm[:], Wt[li][k][:], tts[k][:],
                                         start=(k == 0), stop=(k == 2))
                    ot = smallp.tile([HID, P], f32, tag="ot")
                    nc.scalar.activation(ot[:], pm[:],
                                         mybir.ActivationFunctionType.Relu,
                                         bias=bt[li][:])
                    # transpose back to node-major
                    pt2 = psump.tile([P, HID], f32, tag="tb")
                    nc.tensor.transpose(out=pt2[:], in_=ot[:],
                                        identity=ident[:HID, :HID])
                    nc.scalar.copy(out=out_tile[:, t, :], in_=pt2[:])

            table = x_table.ap()
            agi = 0
            for li in range(3):
                is_last = li == 2
                spmm(table, tx1)
                t1tab = allgather(tx1, agi); agi += 1
                spmm(t1tab, tx2)
                # tx2 = 2*L(tx1) - tx0
                nc.vector.tensor_scalar_mul(tx2[:], tx2[:], 2.0)
                nc.vector.tensor_tensor(out=tx2[:], in0=tx2[:], in1=tx0[:],
                                        op=AO.subtract)
                dense([tx0, tx1, tx2], li, tx0, is_last)
                if not is_last:
                    table = allgather(tx0, agi); agi += 1

            # log_softmax over first F_OUT_REAL features (rest of HID cols
            # are padding: W2/b2 padded with -inf? -> handled by masking)
            lg = tx0
            mx = smallp.tile([P, NT, 1], f32, tag="mx")
            nc.vector.tensor_reduce(out=mx[:], in_=lg[:, :, :F_OUT_REAL],
                                    axis=mybir.AxisListType.X, op=AO.max)
            sh = txp.tile([P, NT, F_OUT_REAL], f32, tag="sh")
            nc.vector.tensor_tensor(
                out=sh[:], in0=lg[:, :, :F_OUT_REAL],
                in1=mx[:].to_broadcast([P, NT, F_OUT_REAL]), op=AO.subtract)
            ex = txp.tile([P, NT, F_OUT_REAL], f32, tag="ex")
            nc.scalar.activation(ex[:], sh[:],
                                 mybir.ActivationFunctionType.Exp)
            sm = smallp.tile([P, NT, 1], f32, tag="sm")
            nc.vector.tensor_reduce(out=sm[:], in_=ex[:],
                                    axis=mybir.AxisListType.X, op=AO.add)
            lz = smallp.tile([P, NT, 1], f32, tag="lz")
            nc.scalar.activation(lz[:], sm[:], mybir.ActivationFunctionType.Ln)
            nc.vector.tensor_tensor(
                out=sh[:], in0=sh[:],
                in1=lz[:].to_broadcast([P, NT, F_OUT_REAL]), op=AO.subtract)
            out_t = tx1
            nc.vector.memset(out_t[:], 0.0)
            nc.vector.tensor_copy(out=out_t[:, :, :F_OUT_REAL], in_=sh[:])
            nc.sync.dma_start(
                out=yout.ap().rearrange("(a p) d -> p a d", p=P), in_=out_t[:])
    nc.compile()
    return nc


def kernel(x, edge_index, edge_attr, W0, b0, W1, b1, W2, b2):
    x = np.asarray(x)
    edge_index = np.asarray(edge_index)
    edge_attr = np.asarray(edge_attr)
    key = hash((edge_index.tobytes(), edge_attr.tobytes()))
    if key in _CACHE:
        nc, prep = _CACHE[key]
    else:
        prep = _host_prep(edge_index, edge_attr)
        nc = _build(prep)
        _CACHE[key] = (nc, prep)

    # pad weights/bias to HID=64 wide
    W2p = np.zeros((3, HID, HID), dtype=np.float32)
    W2p[:, :, :F_OUT_REAL] = np.asarray(W2, dtype=np.float32)
    b2p = np.zeros((HID,), dtype=np.float32)
    b2p[:F_OUT_REAL] = np.asarray(b2, dtype=np.float32)

    xpad = np.zeros((TROWS, F_IN), dtype=np.float32)
    xpad[:N_REAL] = np.asarray(x, dtype=np.float32)

    in_maps = []
    for c in range(C):
        in_maps.append({
            "x_own": xpad[c * SROWS:(c + 1) * SROWS],
            "x_table": xpad,
            "gidx": prep["gidx"][c],
            "gnorm": prep["gnorm"][c],
            "ridx": prep["ridx"][c],
            "W0": np.asarray(W0, dtype=np.float32),
            "b0": np.asarray(b0, dtype=np.float32),
            "W1": np.asarray(W1, dtype=np.float32),
            "b1": np.asarray(b1, dtype=np.float32),
            "W2": W2p, "b2": b2p,
        })
    res = run_bass_kernel_spmd(nc, in_maps, core_ids=list(range(C)),
                               trace=TRACE[0])
    LAST_EXEC_NS[0] = res.exec_time_ns
    out = np.concatenate([res.results[c]["yout"] for c in range(C)], axis=0)
    return out[:N_REAL, :F_OUT_REAL].astype(np.float32)


# revision 10
# speedup vs baseline: 2.5020x; 1.2790x over previous
"""ChebConv GNN (K=3, 3 layers) distributed Bass kernel for 8 NeuronCores.

kernel(**inputs) takes FULL numpy inputs (as in setup_inputs) and returns
the FULL [N, 40] float32 log_softmax output.

Design:
- Nodes sharded contiguously across 8 cores (12544 padded rows/core).
- SpMM via dma_gather from a replicated HBM feature table: per core,
  edges bucketed into 4 col-banks (25088-row int16 windows); within a
  bank, local rows are sorted by bank-degree and edges arranged in
  slabs (k-th bank-edge per row) so gather position == accumulator
  slot; per-edge norm applied via DVE broadcast-mul; slab adds on DVE;
  bank partials recombined with static-index gathers + adds.
- Slab schedule padded to a core-uniform profile (SPMD: one program).
- AllGather refreshes the replicated table after each SpMM producer.
- Dense 64x64 matmuls run feature-major on TensorE via PE transposes;
  bias+ReLU on ScalarE; log_softmax fused wide on DVE/ACT.
"""

import numpy as np

import concourse.bacc as bacc
import concourse.mybir as mybir
import concourse.tile as tile
from concourse.bass_utils import run_bass_kernel_spmd

C = 8            # cores
P = 128
SROWS = 12544    # rows per core (98 * 128)
NBANK = 4
# per-core quarter sizes (128-multiples summing to SROWS); table is laid
# out (quarter, core, local) so each quarter is one AllGather chunk and
# one int16 gather bank window (size QSIZE*C < 32768).
QSIZES = [3200, 3200, 3072, 3072]
QSTART = [0, 3200, 6400, 9472]
BANKROWS = [q * C for q in QSIZES]
BBASE = [0, 25600, 51200, 75776]
TROWS = C * SROWS          # padded table rows = 100352
N_REAL = 100000
F_IN = 64
HID = 64
F_OUT_REAL = 40
NT = SROWS // P  # 98 node tiles per core
MAXCALL = 4096   # idxs per dma_gather call

TRACE = [False]
LAST_EXEC_NS = [None]
_CACHE = {}


def _wrap_idx(idx):
    """dma_gather idx layout [128, len/16] int16: position j ->
    (partition j%16, slot j//16), replicated across 8 Q7 core groups."""
    n = len(idx)
    a = idx.astype(np.int16).reshape(n // 16, 16).T
    return np.broadcast_to(a[None], (8, 16, n // 16)).reshape(P, n // 16)


def _host_prep(edge_index, edge_attr):
    row = edge_index[0].astype(np.int64)
    col = edge_index[1].astype(np.int64)
    w = edge_attr.astype(np.float64)
    deg = np.zeros(N_REAL)
    np.add.at(deg, row, w)
    dinv = np.where(deg > 0, deg ** -0.5, 0.0)
    norm = (-(dinv[row] * w * dinv[col])).astype(np.float32)

    per = [[None] * NBANK for _ in range(C)]
    shard = row // SROWS
    cc = col // SROWS
    jj = col % SROWS
    bank = np.zeros(len(col), dtype=np.int64)
    blocal = np.zeros(len(col), dtype=np.int64)
    for q in range(NBANK):
        mq = (jj >= QSTART[q]) & (jj < QSTART[q] + QSIZES[q])
        bank[mq] = q
        blocal[mq] = cc[mq] * QSIZES[q] + (jj[mq] - QSTART[q])
    for c in range(C):
        mc = shard == c
        for b in range(NBANK):
            m = mc & (bank == b)
            er = row[m] - c * SROWS
            ec = blocal[m]
            en = norm[m]
            bdeg = np.bincount(er, minlength=SROWS)
            order = np.argsort(-bdeg, kind="stable")   # slot -> row
            rank = np.empty(SROWS, dtype=np.int64)     # row -> slot
            rank[order] = np.arange(SROWS)
            sdeg = bdeg[order]
            maxd = int(sdeg[0]) if len(er) else 0
            lens = [int((sdeg > k).sum()) for k in range(maxd)]
            eslot = rank[er]
            o1 = np.argsort(eslot, kind="stable")
            es = eslot[o1]
            kidx = np.arange(len(es)) - np.searchsorted(es, es)
            o2 = np.lexsort((es, kidx))
            eorder = o1[o2]
            # edges now ordered (k, slot); within slab k, position = slot
            per[c][b] = dict(lens=lens, eslot=eslot[eorder], ecol=ec[eorder],
                             enorm=en[eorder], rank=rank)

    profile = []
    for b in range(NBANK):
        nk = max(len(per[c][b]["lens"]) for c in range(C))
        plens = []
        for k in range(nk):
            L = max((per[c][b]["lens"][k] if k < len(per[c][b]["lens"]) else 0)
                    for c in range(C))
            plens.append(max(P, -(-L // P) * P))
        profile.append(plens)
    totpos = sum(sum(pl) for pl in profile)

    gidx = np.zeros((C, P, totpos // 16), dtype=np.int16)
    gnorm = np.zeros((C, P, totpos // P), dtype=np.float32)
    # call = (bank, idx16_off, gnorm_slot_off, acc_slot_off, num_idx)
    calls = []
    off = 0
    for b in range(NBANK):
        for k, L in enumerate(profile[b]):
            pos0 = off
            for c in range(C):
                d = per[c][b]
                idx = np.zeros(L, dtype=np.int64)
                nrm = np.zeros(L, dtype=np.float32)
                if k < len(d["lens"]):
                    lk = d["lens"][k]
                    s0 = sum(d["lens"][:k])
                    sl = d["eslot"][s0:s0 + lk]
                    idx[sl] = d["ecol"][s0:s0 + lk]
                    nrm[sl] = d["enorm"][s0:s0 + lk]
                gnorm[c][:, pos0 // P:(pos0 + L) // P] = nrm.reshape(L // P, P).T
                o = pos0
                for cs in range(0, L, MAXCALL):
                    ni = min(MAXCALL, L - cs)
                    gidx[c][:, o // 16:(o + ni) // 16] = _wrap_idx(idx[cs:cs + ni])
                    o += ni
            for cs in range(0, L, MAXCALL):
                ni = min(MAXCALL, L - cs)
                calls.append((b, (pos0 + cs) // 16, (pos0 + cs) // P, cs // P, ni))
            off += L

    ridx = np.zeros((C, NBANK, P, SROWS // 16), dtype=np.int16)
    for b in range(NBANK):
        for c in range(C):
            rk = per[c][b]["rank"]
            for q in range(NBANK):
                cs, ni = QSTART[q], QSIZES[q]
                ridx[c][b][:, cs // 16:(cs + ni) // 16] = _wrap_idx(rk[cs:cs + ni])
    # recombine calls grouped by quarter: (quarter, idx16_off, slot_off, ni)
    rcalls = [(q, QSTART[q] // 16, QSTART[q] // P, QSIZES[q])
              for q in range(NBANK)]
    return dict(gidx=gidx, gnorm=gnorm, ridx=ridx, calls=calls, rcalls=rcalls,
                totpos=totpos)


def _build(prep):
    totpos = prep["totpos"]
    calls = prep["calls"]
    rcalls = prep["rcalls"]
    f32 = mybir.dt.float32
    i16 = mybir.dt.int16
    AO = mybir.AluOpType

    nc = bacc.Bacc("TRN2", target_bir_lowering=False, debug=False, num_devices=C,
                   num_swdge_queues=4)
    x_own = nc.declare_dram_parameter("x_own", [SROWS, F_IN], f32, isOutput=False)
    x_table = nc.declare_dram_parameter("x_table", [TROWS, F_IN], f32, isOutput=False)
    gidx_d = nc.declare_dram_parameter("gidx", [P, totpos // 16], i16, isOutput=False)
    gnorm_d = nc.declare_dram_parameter("gnorm", [P, totpos // P], f32, isOutput=False)
    ridx_d = nc.declare_dram_parameter("ridx", [NBANK, P, SROWS // 16], i16, isOutput=False)
    Wd, bd = [], []
    for i in range(3):
        Wd.append(nc.declare_dram_parameter(f"W{i}", [3, HID, HID], f32, isOutput=False))
        bd.append(nc.declare_dram_parameter(f"b{i}", [HID], f32, isOutput=False))
    yout = nc.declare_dram_parameter("yout", [SROWS, HID], f32, isOutput=True)

    accb = nc.dram_tensor("accb", [NBANK, SROWS, HID], f32)
    agin = [[nc.dram_tensor(f"agin{i}_{q}", [QSIZES[q], HID], f32)
             for q in range(NBANK)] for i in range(5)]
    agout = [[nc.dram_tensor(f"agout{i}_{q}", [BANKROWS[q], HID], f32,
                             addr_space="Shared")
              for q in range(NBANK)] for i in range(5)]

    with tile.TileContext(nc) as tc:
        with (
            tc.tile_pool(name="res", bufs=1) as res,
            tc.tile_pool(name="stage", bufs=4) as stagep,
            tc.tile_pool(name="accp", bufs=1) as accp,
            tc.tile_pool(name="idxp", bufs=6) as idxp,
            tc.tile_pool(name="small", bufs=4) as smallp,
            tc.tile_pool(name="psum", bufs=2, space="PSUM") as psump,
            tc.tile_pool(name="txp", bufs=1) as txp,
        ):
            from concourse.masks import make_identity
            ident = res.tile([P, P], f32)
            make_identity(nc, ident[:])
            gnorm_t = res.tile([P, totpos // P], f32)
            nc.sync.dma_start(out=gnorm_t[:], in_=gnorm_d[:, :])
            Wt, bt = [], []
            for i in range(3):
                ws = []
                for k in range(3):
                    t = res.tile([HID, HID], f32, tag=f"w{i}{k}")
                    nc.sync.dma_start(out=t[:], in_=Wd[i][k])
                    ws.append(t)
                Wt.append(ws)
                t = res.tile([HID, 1], f32, tag=f"bb{i}")
                nc.sync.dma_start(out=t[:], in_=bd[i][:, None])
                bt.append(t)

            tx0 = txp.tile([P, NT, HID], f32, tag="tx0")
            tx1 = txp.tile([P, NT, HID], f32, tag="tx1")
            tx2 = txp.tile([P, NT, HID], f32, tag="tx2")
            nc.sync.dma_start(out=tx0[:], in_=x_own.ap().rearrange("(a p) d -> p a d", p=P))

            qctr = [0]

            def spmm(tables, out_tile, on_quarter=None):
                """out_tile[128, NT, HID] node-major canonical = L @ table.
                tables: list of NBANK bank-window DRAM APs."""
                for b in range(NBANK):
                    acc = accp.tile([P, NT, HID], f32, tag="acc")
                    nc.vector.memset(acc[:], 0.0)
                    for (bb, i16o, so, ao, ni) in calls:
                        if bb != b:
                            continue
                        st = stagep.tile([P, MAXCALL // P, HID], f32, tag="st")
                        it = idxp.tile([P, MAXCALL // 16], i16, tag="it")
                        nc.sync.dma_start(out=it[:, :ni // 16],
                                          in_=gidx_d[:, i16o:i16o + ni // 16])
                        nc.gpsimd.dma_gather(
                            st[:, :ni // P, :],
                            tables[b],
                            it[:, :ni // 16], ni, ni, HID,
                            single_packet=False,
                            queue_num=qctr[0] % 4,
                        )
                        qctr[0] += 1
                        nrm_b = gnorm_t[:, so:so + ni // P, None].to_broadcast(
                            [P, ni // P, HID])
                        nc.vector.tensor_tensor(out=st[:, :ni // P, :],
                                                in0=st[:, :ni // P, :],
                                                in1=nrm_b, op=AO.mult)
                        nc.vector.tensor_tensor(
                            out=acc[:, ao:ao + ni // P, :],
                            in0=acc[:, ao:ao + ni // P, :],
                            in1=st[:, :ni // P, :], op=AO.add)
                    nc.sync.dma_start(
                        out=accb.ap()[b].rearrange("(a p) d -> p a d", p=P),
                        in_=acc[:])
                for (q, i16o, so, ni) in rcalls:
                    for b in range(NBANK):
                        it = idxp.tile([P, MAXCALL // 16], i16, tag="it")
                        nc.sync.dma_start(out=it[:, :ni // 16],
                                          in_=ridx_d[b][:, i16o:i16o + ni // 16])
                        rst = stagep.tile([P, MAXCALL // P, HID], f32, tag="st")
                        nc.gpsimd.dma_gather(
                            rst[:, :ni // P, :], accb.ap()[b],
                            it[:, :ni // 16], ni, ni, HID,
                            single_packet=False,
                            queue_num=qctr[0] % 4,
                        )
                        qctr[0] += 1
                        if b == 0:
                            nc.vector.tensor_copy(out=out_tile[:, so:so + ni // P, :],
                                                  in_=rst[:, :ni // P, :])
                        else:
                            nc.vector.tensor_tensor(
                                out=out_tile[:, so:so + ni // P, :],
                                in0=out_tile[:, so:so + ni // P, :],
                                in1=rst[:, :ni // P, :], op=AO.add)
                    if on_quarter is not None:
                        on_quarter(q)

            def ag_quarter(src_tile, i, q):
                t0, nt = QSTART[q] // P, QSIZES[q] // P
                nc.sync.dma_start(
                    out=agin[i][q].ap().rearrange("(a p) d -> p a d", p=P),
                    in_=src_tile[:, t0:t0 + nt, :])
                nc.gpsimd.collective_compute(
                    "AllGather", AO.bypass,
                    replica_groups=[list(range(C))],
                    ins=[agin[i][q].ap().opt()], outs=[agout[i][q].ap().opt()],
                )

            def ag_tables(i):
                return [agout[i][q].ap() for q in range(NBANK)]

            def transpose_tile(src):  # [128, 64] sbuf -> [64, 128] sbuf
                pt = psump.tile([HID, P], f32, tag="tp")
                nc.tensor.transpose(out=pt[:], in_=src, identity=ident[:])
                st = smallp.tile([HID, P], f32, tag="tps")
                nc.scalar.copy(out=st[:], in_=pt[:])
                return st

            def dense(txs, li, out_tile, is_last):
                """out_tile[128, NT, HID] = act(sum_k txs[k] @ W[li][k] + b)."""
                for t in range(NT):
                    tts = [transpose_tile(tx[:, t, :]) for tx in txs]
                    pm = psump.tile([HID, P], f32, tag="mm")
                    for k in range(3):
                        nc.tensor.matmul(pm[:], Wt[li][k][:], tts[k][:],
                                         start=(k == 0), stop=(k == 2))
                    ot = smallp.tile([HID, P], f32, tag="ot")
                    nc.scalar.activation(ot[:], pm[:],
                                         mybir.ActivationFunctionType.Relu,
                                         bias=bt[li][:])
                    # transpose back to node-major
                    pt2 = psump.tile([P, HID], f32, tag="tb")
                    nc.tensor.transpose(out=pt2[:], in_=ot[:],
                                        identity=ident[:HID, :HID])
                    nc.scalar.copy(out=out_tile[:, t, :], in_=pt2[:])

            tables = [x_table.ap()[BBASE[b]:BBASE[b] + BANKROWS[b], :]
                      for b in range(NBANK)]
            agi = [0]
            for li in range(3):
                is_last = li == 2
                i1 = agi[0]; agi[0] += 1
                spmm(tables, tx1, on_quarter=lambda q, i=i1: ag_quarter(tx1, i, q))
                spmm(ag_tables(i1), tx2)
                # tx2 = 2*L(tx1) - tx0
                nc.vector.tensor_scalar_mul(tx2[:], tx2[:], 2.0)
                nc.vector.tensor_tensor(out=tx2[:], in0=tx2[:], in1=tx0[:],
                                        op=AO.subtract)
                dense([tx0, tx1, tx2], li, tx0, is_last)
                if not is_last:
                    i2 = agi[0]; agi[0] += 1
                    for q in range(NBANK):
                        ag_quarter(tx0, i2, q)
                    tables = ag_tables(i2)

            # log_softmax over first F_OUT_REAL features (rest of HID cols
            # are padding: W2/b2 padded with -inf? -> handled by masking)
            lg = tx0
            mx = smallp.tile([P, NT, 1], f32, tag="mx")
            nc.vector.tensor_reduce(out=mx[:], in_=lg[:, :, :F_OUT_REAL],
                                    axis=mybir.AxisListType.X, op=AO.max)
            sh = txp.tile([P, NT, F_OUT_REAL], f32, tag="sh")
            nc.vector.tensor_tensor(
                out=sh[:], in0=lg[:, :, :F_OUT_REAL],
                in1=mx[:].to_broadcast([P, NT, F_OUT_REAL]), op=AO.subtract)
            ex = txp.tile([P, NT, F_OUT_REAL], f32, tag="ex")
            nc.scalar.activation(ex[:], sh[:],
                                 mybir.ActivationFunctionType.Exp)
            sm = smallp.tile([P, NT, 1], f32, tag="sm")
            nc.vector.tensor_reduce(out=sm[:], in_=ex[:],
                                    axis=mybir.AxisListType.X, op=AO.add)
            lz = smallp.tile([P, NT, 1], f32, tag="lz")
            nc.scalar.activation(lz[:], sm[:], mybir.ActivationFunctionType.Ln)
            nc.vector.tensor_tensor(
                out=sh[:], in0=sh[:],
                in1=lz[:].to_broadcast([P, NT, F_OUT_REAL]), op=AO.subtract)
            out_t = tx1
            nc.vector.memset(out_t[:], 0.0)
            nc.vector.tensor_copy(out=out_t[:, :, :F_OUT_REAL], in_=sh[:])
            nc.sync.dma_start(
                out=yout.ap().rearrange("(a p) d -> p a d", p=P), in_=out_t[:])
    nc.compile()
    return nc


def kernel(x, edge_index, edge_attr, W0, b0, W1, b1, W2, b2):
    x = np.asarray(x)
    edge_index = np.asarray(edge_index)
    edge_attr = np.asarray(edge_attr)
    key = hash((edge_index.tobytes(), edge_attr.tobytes()))
    if key in _CACHE:
        nc, prep = _CACHE[key]
    else:
        prep = _host_prep(edge_index, edge_attr)
        nc = _build(prep)
        _CACHE[key] = (nc, prep)

    # pad weights/bias to HID=64 wide
    W2p = np.zeros((3, HID, HID), dtype=np.float32)
    W2p[:, :, :F_OUT_REAL] = np.asarray(W2, dtype=np.float32)
    b2p = np.zeros((HID,), dtype=np.float32)
    b2p[:F_OUT_REAL] = np.asarray(b2, dtype=np.float32)

    xpad = np.zeros((TROWS, F_IN), dtype=np.float32)
    xpad[:N_REAL] = np.asarray(x, dtype=np.float32)
    # table layout: (quarter, core, local-within-quarter)
    xtab = np.zeros((TROWS, F_IN), dtype=np.float32)
    for q in range(NBANK):
        for c in range(C):
            src0 = c * SROWS + QSTART[q]
            dst0 = BBASE[q] + c * QSIZES[q]
            xtab[dst0:dst0 + QSIZES[q]] = xpad[src0:src0 + QSIZES[q]]

    in_maps = []
    for c in range(C):
        in_maps.append({
            "x_own": xpad[c * SROWS:(c + 1) * SROWS],
            "x_table": xtab,
            "gidx": prep["gidx"][c],
            "gnorm": prep["gnorm"][c],
            "ridx": prep["ridx"][c],
            "W0": np.asarray(W0, dtype=np.float32),
            "b0": np.asarray(b0, dtype=np.float32),
            "W1": np.asarray(W1, dtype=np.float32),
            "b1": np.asarray(b1, dtype=np.float32),
            "W2": W2p, "b2": b2p,
        })
    res = run_bass_kernel_spmd(nc, in_maps, core_ids=list(range(C)),
                               trace=TRACE[0])
    LAST_EXEC_NS[0] = res.exec_time_ns
    out = np.concatenate([res.results[c]["yout"] for c in range(C)], axis=0)
    return out[:N_REAL, :F_OUT_REAL].astype(np.float32)


# revision 12
# speedup vs baseline: 3.0976x; 1.2381x over previous
"""ChebConv GNN (K=3, 3 layers) distributed Bass kernel for 8 NeuronCores.

kernel(**inputs) takes FULL numpy inputs (as in setup_inputs) and returns
the FULL [N, 40] float32 log_softmax output.

Design:
- Nodes sharded contiguously across 8 cores (12544 padded rows/core).
- SpMM via dma_gather from a replicated HBM feature table: per core,
  edges bucketed into 4 col-banks (25088-row int16 windows); within a
  bank, local rows are sorted by bank-degree and edges arranged in
  slabs (k-th bank-edge per row) so gather position == accumulator
  slot; per-edge norm applied via DVE broadcast-mul; slab adds on DVE;
  bank partials recombined with static-index gathers + adds.
- Slab schedule padded to a core-uniform profile (SPMD: one program).
- AllGather refreshes the replicated table after each SpMM producer.
- Dense 64x64 matmuls run feature-major on TensorE via PE transposes;
  bias+ReLU on ScalarE; log_softmax fused wide on DVE/ACT.
"""

import numpy as np

import concourse.bacc as bacc
import concourse.mybir as mybir
import concourse.tile as tile
from concourse.bass_utils import run_bass_kernel_spmd

C = 8            # cores
P = 128
SROWS = 12544    # rows per core (98 * 128)
NBANK = 4
# per-core quarter sizes (128-multiples summing to SROWS); table is laid
# out (quarter, core, local) so each quarter is one AllGather chunk and
# one int16 gather bank window (size QSIZE*C < 32768).
QSIZES = [3200, 3200, 3072, 3072]
QSTART = [0, 3200, 6400, 9472]
BANKROWS = [q * C for q in QSIZES]
BBASE = [0, 25600, 51200, 75776]
TROWS = C * SROWS          # padded table rows = 100352
N_REAL = 100000
F_IN = 64
HID = 64
F_OUT_REAL = 40
NT = SROWS // P  # 98 node tiles per core
MAXCALL = 2048   # idxs per dma_gather call
MAXSTG = 3200    # stage tile capacity (recombine quarters up to 3200)

TRACE = [False]
LAST_EXEC_NS = [None]
_CACHE = {}


def _wrap_idx(idx):
    """dma_gather idx layout [128, len/16] int16: position j ->
    (partition j%16, slot j//16), replicated across 8 Q7 core groups."""
    n = len(idx)
    a = idx.astype(np.int16).reshape(n // 16, 16).T
    return np.broadcast_to(a[None], (8, 16, n // 16)).reshape(P, n // 16)


def _host_prep(edge_index, edge_attr):
    row = edge_index[0].astype(np.int64)
    col = edge_index[1].astype(np.int64)
    w = edge_attr.astype(np.float64)
    deg = np.zeros(N_REAL)
    np.add.at(deg, row, w)
    dinv = np.where(deg > 0, deg ** -0.5, 0.0)
    norm = (-(dinv[row] * w * dinv[col])).astype(np.float32)

    per = [[None] * NBANK for _ in range(C)]
    shard = row // SROWS
    cc = col // SROWS
    jj = col % SROWS
    bank = np.zeros(len(col), dtype=np.int64)
    blocal = np.zeros(len(col), dtype=np.int64)
    for q in range(NBANK):
        mq = (jj >= QSTART[q]) & (jj < QSTART[q] + QSIZES[q])
        bank[mq] = q
        blocal[mq] = cc[mq] * QSIZES[q] + (jj[mq] - QSTART[q])
    for c in range(C):
        mc = shard == c
        for b in range(NBANK):
            m = mc & (bank == b)
            er = row[m] - c * SROWS
            ec = blocal[m]
            en = norm[m]
            bdeg = np.bincount(er, minlength=SROWS)
            order = np.argsort(-bdeg, kind="stable")   # slot -> row
            rank = np.empty(SROWS, dtype=np.int64)     # row -> slot
            rank[order] = np.arange(SROWS)
            sdeg = bdeg[order]
            maxd = int(sdeg[0]) if len(er) else 0
            lens = [int((sdeg > k).sum()) for k in range(maxd)]
            eslot = rank[er]
            o1 = np.argsort(eslot, kind="stable")
            es = eslot[o1]
            kidx = np.arange(len(es)) - np.searchsorted(es, es)
            o2 = np.lexsort((es, kidx))
            eorder = o1[o2]
            # edges now ordered (k, slot); within slab k, position = slot
            per[c][b] = dict(lens=lens, eslot=eslot[eorder], ecol=ec[eorder],
                             enorm=en[eorder], rank=rank)

    profile = []
    for b in range(NBANK):
        nk = max(len(per[c][b]["lens"]) for c in range(C))
        plens = []
        for k in range(nk):
            L = max((per[c][b]["lens"][k] if k < len(per[c][b]["lens"]) else 0)
                    for c in range(C))
            plens.append(max(P, -(-L // P) * P))
        profile.append(plens)
    totpos = sum(sum(pl) for pl in profile)

    gidx = np.zeros((C, P, totpos // 16), dtype=np.int16)
    gnorm = np.zeros((C, P, totpos // P), dtype=np.float32)
    # call = (bank, idx16_off, gnorm_slot_off, acc_slot_off, num_idx)
    calls = []
    off = 0
    for b in range(NBANK):
        for k, L in enumerate(profile[b]):
            pos0 = off
            for c in range(C):
                d = per[c][b]
                idx = np.zeros(L, dtype=np.int64)
                nrm = np.zeros(L, dtype=np.float32)
                if k < len(d["lens"]):
                    lk = d["lens"][k]
                    s0 = sum(d["lens"][:k])
                    sl = d["eslot"][s0:s0 + lk]
                    idx[sl] = d["ecol"][s0:s0 + lk]
                    nrm[sl] = d["enorm"][s0:s0 + lk]
                gnorm[c][:, pos0 // P:(pos0 + L) // P] = nrm.reshape(L // P, P).T
                o = pos0
                for cs in range(0, L, MAXCALL):
                    ni = min(MAXCALL, L - cs)
                    gidx[c][:, o // 16:(o + ni) // 16] = _wrap_idx(idx[cs:cs + ni])
                    o += ni
            for cs in range(0, L, MAXCALL):
                ni = min(MAXCALL, L - cs)
                calls.append((b, (pos0 + cs) // 16, (pos0 + cs) // P, cs // P, ni))
            off += L

    ridx = np.zeros((C, NBANK, P, SROWS // 16), dtype=np.int16)
    for b in range(NBANK):
        for c in range(C):
            rk = per[c][b]["rank"]
            for q in range(NBANK):
                cs, ni = QSTART[q], QSIZES[q]
                ridx[c][b][:, cs // 16:(cs + ni) // 16] = _wrap_idx(rk[cs:cs + ni])
    # recombine calls grouped by quarter: (quarter, idx16_off, slot_off, ni)
    rcalls = [(q, QSTART[q] // 16, QSTART[q] // P, QSIZES[q])
              for q in range(NBANK)]
    return dict(gidx=gidx, gnorm=gnorm, ridx=ridx, calls=calls, rcalls=rcalls,
                totpos=totpos)


def _build(prep):
    totpos = prep["totpos"]
    calls = prep["calls"]
    rcalls = prep["rcalls"]
    f32 = mybir.dt.float32
    i16 = mybir.dt.int16
    AO = mybir.AluOpType

    nc = bacc.Bacc("TRN2", target_bir_lowering=False, debug=False, num_devices=C,
                   num_swdge_queues=4)
    x_own = nc.declare_dram_parameter("x_own", [SROWS, F_IN], f32, isOutput=False)
    x_table = nc.declare_dram_parameter("x_table", [TROWS, F_IN], f32, isOutput=False)
    gidx_d = nc.declare_dram_parameter("gidx", [P, totpos // 16], i16, isOutput=False)
    gnorm_d = nc.declare_dram_parameter("gnorm", [P, totpos // P], f32, isOutput=False)
    ridx_d = nc.declare_dram_parameter("ridx", [NBANK, P, SROWS // 16], i16, isOutput=False)
    Wd, bd = [], []
    for i in range(3):
        Wd.append(nc.declare_dram_parameter(f"W{i}", [3, HID, HID], f32, isOutput=False))
        bd.append(nc.declare_dram_parameter(f"b{i}", [HID], f32, isOutput=False))
    yout = nc.declare_dram_parameter("yout", [SROWS, HID], f32, isOutput=True)

    accb = nc.dram_tensor("accb", [NBANK, SROWS, HID], f32)
    agin = [[nc.dram_tensor(f"agin{i}_{q}", [QSIZES[q], HID], f32)
             for q in range(NBANK)] for i in range(5)]
    agout = [[nc.dram_tensor(f"agout{i}_{q}", [BANKROWS[q], HID], f32,
                             addr_space="Shared")
              for q in range(NBANK)] for i in range(5)]

    with tile.TileContext(nc) as tc:
        with (
            tc.tile_pool(name="res", bufs=1) as res,
            tc.tile_pool(name="stage", bufs=6) as stagep,
            tc.tile_pool(name="accp", bufs=1) as accp,
            tc.tile_pool(name="idxp", bufs=8) as idxp,
            tc.tile_pool(name="small", bufs=4) as smallp,
            tc.tile_pool(name="psum", bufs=2, space="PSUM") as psump,
            tc.tile_pool(name="txp", bufs=1) as txp,
        ):
            from concourse.masks import make_identity
            ident = res.tile([P, P], f32)
            make_identity(nc, ident[:])
            gnorm_t = res.tile([P, totpos // P], f32)
            nc.sync.dma_start(out=gnorm_t[:], in_=gnorm_d[:, :])
            Wt, bt = [], []
            for i in range(3):
                ws = []
                for k in range(3):
                    t = res.tile([HID, HID], f32, tag=f"w{i}{k}")
                    nc.sync.dma_start(out=t[:], in_=Wd[i][k])
                    ws.append(t)
                Wt.append(ws)
                t = res.tile([HID, 1], f32, tag=f"bb{i}")
                nc.sync.dma_start(out=t[:], in_=bd[i][:, None])
                bt.append(t)

            tx0 = txp.tile([P, NT, HID], f32, tag="tx0")
            tx1 = txp.tile([P, NT, HID], f32, tag="tx1")
            tx2 = txp.tile([P, NT, HID], f32, tag="tx2")
            nc.sync.dma_start(out=tx0[:], in_=x_own.ap().rearrange("(a p) d -> p a d", p=P))

            qctr = [0]

            def spmm(tables, out_tile, on_quarter=None):
                """out_tile[128, NT, HID] node-major canonical = L @ table.
                tables: list of NBANK bank-window DRAM APs."""
                for b in range(NBANK):
                    acc = accp.tile([P, NT, HID], f32, tag="acc")
                    nc.vector.memset(acc[:], 0.0)
                    for (bb, i16o, so, ao, ni) in calls:
                        if bb != b:
                            continue
                        st = stagep.tile([P, MAXSTG // P, HID], f32, tag="st")
                        it = idxp.tile([P, MAXSTG // 16], i16, tag="it")
                        nc.sync.dma_start(out=it[:, :ni // 16],
                                          in_=gidx_d[:, i16o:i16o + ni // 16])
                        nc.gpsimd.dma_gather(
                            st[:, :ni // P, :],
                            tables[b],
                            it[:, :ni // 16], ni, ni, HID,
                            single_packet=False,
                            queue_num=qctr[0] % 4,
                        )
                        qctr[0] += 1
                        nrm_b = gnorm_t[:, so:so + ni // P, None].to_broadcast(
                            [P, ni // P, HID])
                        nc.vector.tensor_tensor(out=st[:, :ni // P, :],
                                                in0=st[:, :ni // P, :],
                                                in1=nrm_b, op=AO.mult)
                        nc.vector.tensor_tensor(
                            out=acc[:, ao:ao + ni // P, :],
                            in0=acc[:, ao:ao + ni // P, :],
                            in1=st[:, :ni // P, :], op=AO.add)
                    nc.sync.dma_start(
                        out=accb.ap()[b].rearrange("(a p) d -> p a d", p=P),
                        in_=acc[:])
                for (q, i16o, so, ni) in rcalls:
                    for b in range(NBANK):
                        it = idxp.tile([P, MAXSTG // 16], i16, tag="it")
                        nc.sync.dma_start(out=it[:, :ni // 16],
                                          in_=ridx_d[b][:, i16o:i16o + ni // 16])
                        rst = stagep.tile([P, MAXSTG // P, HID], f32, tag="st")
                        nc.gpsimd.dma_gather(
                            rst[:, :ni // P, :], accb.ap()[b],
                            it[:, :ni // 16], ni, ni, HID,
                            single_packet=False,
                            queue_num=qctr[0] % 4,
                        )
                        qctr[0] += 1
                        if b == 0:
                            nc.vector.tensor_copy(out=out_tile[:, so:so + ni // P, :],
                                                  in_=rst[:, :ni // P, :])
                        else:
                            nc.vector.tensor_tensor(
                                out=out_tile[:, so:so + ni // P, :],
                                in0=out_tile[:, so:so + ni // P, :],
                                in1=rst[:, :ni // P, :], op=AO.add)
                    if on_quarter is not None:
                        on_quarter(q)

            def ag_quarter(src_tile, i, q):
                t0, nt = QSTART[q] // P, QSIZES[q] // P
                nc.sync.dma_start(
                    out=agin[i][q].ap().rearrange("(a p) d -> p a d", p=P),
                    in_=src_tile[:, t0:t0 + nt, :])
                nc.gpsimd.collective_compute(
                    "AllGather", AO.bypass,
                    replica_groups=[list(range(C))],
                    ins=[agin[i][q].ap().opt()], outs=[agout[i][q].ap().opt()],
                )

            def ag_tables(i):
                return [agout[i][q].ap() for q in range(NBANK)]

            def transpose_tile(src):  # [128, 64] sbuf -> [64, 128] sbuf
                pt = psump.tile([HID, P], f32, tag="tp")
                nc.tensor.transpose(out=pt[:], in_=src, identity=ident[:])
                st = smallp.tile([HID, P], f32, tag="tps")
                nc.scalar.copy(out=st[:], in_=pt[:])
                return st

            def dense(txs, li, out_tile, is_last):
                """out_tile[128, NT, HID] = act(sum_k txs[k] @ W[li][k] + b)."""
                for t in range(NT):
                    tts = [transpose_tile(tx[:, t, :]) for tx in txs]
                    pm = psump.tile([HID, P], f32, tag="mm")
                    for k in range(3):
                        nc.tensor.matmul(pm[:], Wt[li][k][:], tts[k][:],
                                         start=(k == 0), stop=(k == 2))
                    ot = smallp.tile([HID, P], f32, tag="ot")
                    nc.scalar.activation(ot[:], pm[:],
                                         mybir.ActivationFunctionType.Relu,
                                         bias=bt[li][:])
                    # transpose back to node-major
                    pt2 = psump.tile([P, HID], f32, tag="tb")
                    nc.tensor.transpose(out=pt2[:], in_=ot[:],
                                        identity=ident[:HID, :HID])
                    nc.scalar.copy(out=out_tile[:, t, :], in_=pt2[:])

            tables = [x_table.ap()[BBASE[b]:BBASE[b] + BANKROWS[b], :]
                      for b in range(NBANK)]
            agi = [0]
            for li in range(3):
                is_last = li == 2
                i1 = agi[0]; agi[0] += 1
                spmm(tables, tx1, on_quarter=lambda q, i=i1: ag_quarter(tx1, i, q))
                spmm(ag_tables(i1), tx2)
                # tx2 = 2*L(tx1) - tx0
                nc.vector.tensor_scalar_mul(tx2[:], tx2[:], 2.0)
                nc.vector.tensor_tensor(out=tx2[:], in0=tx2[:], in1=tx0[:],
                                        op=AO.subtract)
                dense([tx0, tx1, tx2], li, tx0, is_last)
                if not is_last:
                    i2 = agi[0]; agi[0] += 1
                    for q in range(NBANK):
                        ag_quarter(tx0, i2, q)
                    tables = ag_tables(i2)

            # log_softmax over first F_OUT_REAL features (rest of HID cols
            # are padding: W2/b2 padded with -inf? -> handled by masking)
            lg = tx0
            mx = smallp.tile([P, NT, 1], f32, tag="mx")
            nc.vector.tensor_reduce(out=mx[:], in_=lg[:, :, :F_OUT_REAL],
                                    axis=mybir.AxisListType.X, op=AO.max)
            sh = txp.tile([P, NT, F_OUT_REAL], f32, tag="sh")
            nc.vector.tensor_tensor(
                out=sh[:], in0=lg[:, :, :F_OUT_REAL],
                in1=mx[:].to_broadcast([P, NT, F_OUT_REAL]), op=AO.subtract)
            ex = txp.tile([P, NT, F_OUT_REAL], f32, tag="ex")
            nc.scalar.activation(ex[:], sh[:],
                                 mybir.ActivationFunctionType.Exp)
            sm = smallp.tile([P, NT, 1], f32, tag="sm")
            nc.vector.tensor_reduce(out=sm[:], in_=ex[:],
                                    axis=mybir.AxisListType.X, op=AO.add)
            lz = smallp.tile([P, NT, 1], f32, tag="lz")
            nc.scalar.activation(lz[:], sm[:], mybir.ActivationFunctionType.Ln)
            nc.vector.tensor_tensor(
                out=sh[:], in0=sh[:],
                in1=lz[:].to_broadcast([P, NT, F_OUT_REAL]), op=AO.subtract)
            out_t = tx1
            nc.vector.memset(out_t[:], 0.0)
            nc.vector.tensor_copy(out=out_t[:, :, :F_OUT_REAL], in_=sh[:])
            nc.sync.dma_start(
                out=yout.ap().rearrange("(a p) d -> p a d", p=P), in_=out_t[:])
    nc.compile()
    return nc


def kernel(x, edge_index, edge_attr, W0, b0, W1, b1, W2, b2):
    x = np.asarray(x)
    edge_index = np.asarray(edge_index)
    edge_attr = np.asarray(edge_attr)
    key = hash((edge_index.tobytes(), edge_attr.tobytes()))
    if key in _CACHE:
        nc, prep = _CACHE[key]
    else:
        prep = _host_prep(edge_index, edge_attr)
        nc = _build(prep)
        _CACHE[key] = (nc, prep)

    # pad weights/bias to HID=64 wide
    W2p = np.zeros((3, HID, HID), dtype=np.float32)
    W2p[:, :, :F_OUT_REAL] = np.asarray(W2, dtype=np.float32)
    b2p = np.zeros((HID,), dtype=np.float32)
    b2p[:F_OUT_REAL] = np.asarray(b2, dtype=np.float32)

    xpad = np.zeros((TROWS, F_IN), dtype=np.float32)
    xpad[:N_REAL] = np.asarray(x, dtype=np.float32)
    # table layout: (quarter, core, local-within-quarter)
    xtab = np.zeros((TROWS, F_IN), dtype=np.float32)
    for q in range(NBANK):
        for c in range(C):
            src0 = c * SROWS + QSTART[q]
            dst0 = BBASE[q] + c * QSIZES[q]
            xtab[dst0:dst0 + QSIZES[q]] = xpad[src0:src0 + QSIZES[q]]

    in_maps = []
    for c in range(C):
        in_maps.append({
            "x_own": xpad[c * SROWS:(c + 1) * SROWS],
            "x_table": xtab,
            "gidx": prep["gidx"][c],
            "gnorm": prep["gnorm"][c],
            "ridx": prep["ridx"][c],
            "W0": np.asarray(W0, dtype=np.float32),
            "b0": np.asarray(b0, dtype=np.float32),
            "W1": np.asarray(W1, dtype=np.float32),
            "b1": np.asarray(b1, dtype=np.float32),
            "W2": W2p, "b2": b2p,
        })
    res = run_bass_kernel_spmd(nc, in_maps, core_ids=list(range(C)),
                               trace=TRACE[0])
    LAST_EXEC_NS[0] = res.exec_time_ns
    out = np.concatenate([res.results[c]["yout"] for c in range(C)], axis=0)
    return out[:N_REAL, :F_OUT_REAL].astype(np.float32)
